# revision 13
# baseline (speedup 1.0000x reference)
"""Trainium2 Bass kernel for the fused attention module.

8-core sharding: data-parallel over batch (B=2) x tensor-parallel over head
groups (32 heads -> 4 groups of 8). Core c handles batch c//4, head group c%4.
Each core computes QKV projection (its head slice), RoPE, full non-causal
attention for its 8 heads, and a partial output projection against its
W_out column slice; the host sums the 4 partials per batch.

v2 design (vs baseline):
- all matmul operands in bf16 (same PE rate as f32r at N>=256 per the cost
  model, half the DMA/SBUF); rotary math stays f32, one bf16 rounding on the
  stored q/k
- q/k stay resident in SBUF ([128c, 8mf, 2048n] bf16) -- no DRAM spill
- attn@v computed transposed: out[i, d] via lhsT=exp[j, i-tile], rhs=v[j, 65]
  (M=128 instead of 65 -> half the PE time); the 65th v column of ones gives
  the softmax denominator; normalize = per-partition reciprocal broadcast;
  PE-transpose (identity matmul) back to [d, i] for the output projection
- phase CD is software-pipelined ("weave"): per head-pair step, the 8
  score-groups (PE) + exps (Act) are interleaved with the previous pair's
  attn@v chains and the previous i-block's output-projection pieces so the
  Activation engine (CD bottleneck ~255us of exp) never starves.

v3 (vs v2):
- scores exploit PE row tiling: the two heads of a pair contract over
  disjoint 64-partition bands (row_grp h0/h64), and the hardware runs
  such matmuls CONCURRENTLY when adjacent in the engine stream (~3ns
  stagger). v2 separated them because sc0/sc1 had separate psum pools
  freed by two sequential ACTIVATEs; v3 puts all 4 score matmuls of a
  group in ONE 4-bank psum tile consumed by ONE ACTIVATE, so all four
  become schedule-ready together and pair up h0/h64 back-to-back.
- rotary rotate-half matmul in bf16 (was f32r at ~2.7x the cycles); the
  permutation matrix is exact in bf16, input rounding is ~0.4%.
- startup: first-needed DMAs (wqk pair 0 chunked, x block 0) issue ahead
  of the ident/rot-matrix loads so the first projection matmul starts
  sooner.

Orientation notes (PE computes out = lhsT.T @ rhs, contraction on partitions):
- qT/kT produced as [f, n] (lhsT = W slice pre-transposed on host, rhs = xT)
- v produced as [n, f] (lhsT = xT tile, rhs = WvT)
- scoresT[j, i] per head (lhsT = kT j-tile, rhs = qT i-block)
- RoPE rotate_half is a partition shift via a constant permutation matmul
"""

import os
import sys

sys.path.insert(0, "/opt/trn_rl_repo")

import numpy as np
import ml_dtypes

import concourse.bass as bass  # noqa: F401
import concourse.mybir as mybir
import concourse.tile as tile
from concourse import bacc
from concourse.bass import ts
from concourse.bass_utils import run_bass_kernel_spmd

F32 = mybir.dt.float32
F32R = mybir.dt.float32r
F16 = mybir.dt.float16
BF16 = mybir.dt.bfloat16
NPBF16 = ml_dtypes.bfloat16

P = 128
NSEQ = 2048          # sequence length
CDIM = 2048          # model dim
HD = 64              # head dim
NHC = 8              # heads per core
KT = CDIM // P       # 16 contraction tiles
NB = 256             # n-block in the fused projection phase
NNB = NSEQ // NB     # 8
IB = 512             # i-block in attention
NIB = NSEQ // IB     # 4
JT = NSEQ // P       # 16 j-tiles
MF = 8               # q/k feature tiles (0-3 q, 4-7 k)
EXP_FUNC = mybir.ActivationFunctionType.Exp
SCALE = 1.0 / 8.0    # 1/sqrt(HD)

_CACHED_NC = None


def _build_nc():
    nc = bacc.Bacc(None)

    xt = nc.declare_dram_parameter("xt", [NNB, P, KT, NB], BF16, isOutput=False)
    wqkt = nc.declare_dram_parameter("wqkt", [MF, P, KT, P], BF16, isOutput=False)
    wvt = nc.declare_dram_parameter("wvt", [P, KT, 512], BF16, isOutput=False)
    wot = nc.declare_dram_parameter("wot", [P, 4, CDIM], BF16, isOutput=False)
    cos2t = nc.declare_dram_parameter("cos2t", [P, NSEQ], BF16, isOutput=False)
    sin2t = nc.declare_dram_parameter("sin2t", [P, NSEQ], BF16, isOutput=False)
    r2t = nc.declare_dram_parameter("r2t", [P, P], F16, isOutput=False)
    ident = nc.declare_dram_parameter("ident", [P, P], BF16, isOutput=False)
    out_part = nc.declare_dram_parameter("out_part", [NSEQ, CDIM], F32, isOutput=True)

    n_repeat = int(os.environ.get("ATT_REPEAT", "1"))

    with tile.TileContext(nc) as tc, nc.allow_low_precision("bf16 matmul kernel"):
        for _rep in range(n_repeat):
            _kernel_body(nc, tc, xt, wqkt, wvt, wot, cos2t, sin2t, r2t,
                         ident, out_part)

    nc.compile()
    return nc


def _kernel_body(nc, tc, xt, wqkt, wvt, wot, cos2t, sin2t, r2t, ident,
                 out_part):
    with tc.tile_pool(name="persist", bufs=1) as persist:
        qk_sb = persist.tile([P, MF, NSEQ], BF16)
        v1_sb = persist.tile([P, JT, NHC, HD + 1], BF16)
        att_sb = persist.tile([P, 4, NSEQ], BF16)
        wot_sb = persist.tile([P, 4, CDIM], BF16)
        id_sb = persist.tile([P, P], BF16)
        r2_sb = persist.tile([P, P], F16)

        # pools that span pass 0 AND the weave (q/k projection streaming)
        with tc.tile_pool(name="wqkp", bufs=2) as wqkp, \
             tc.tile_pool(name="xtp", bufs=2) as xtp, \
             tc.tile_pool(name="csp", bufs=2) as csp, \
             tc.tile_pool(name="stg", bufs=3) as stg, \
             tc.tile_pool(name="psqr", bufs=1, space="PSUM") as psqr:

            wqk_tiles = {}

            def load_wqk(pair):
                wqk_t = wqkp.tile([P, KT, 2, P], BF16, tag="wqk")
                nc.sync.dma_start(out=wqk_t[:, :, 0, :], in_=wqkt[pair])
                nc.sync.dma_start(out=wqk_t[:, :, 1, :], in_=wqkt[4 + pair])
                wqk_tiles[pair] = wqk_t

            pref = {}

            def load_xcs(nb, first=None):
                if first is not None:
                    xt_t = first
                else:
                    xt_t = xtp.tile([P, KT, NB], BF16, tag="xt")
                    nc.sync.dma_start(out=xt_t, in_=xt[nb])
                cos_sb = csp.tile([P, NB], BF16, tag="cos")
                sin_sb = csp.tile([P, NB], BF16, tag="sin")
                nc.sync.dma_start(out=cos_sb, in_=cos2t[:, ts(nb, NB)])
                nc.sync.dma_start(out=sin_sb, in_=sin2t[:, ts(nb, NB)])
                return xt_t, cos_sb, sin_sb

            def qk_half(pair, nb, qk):
                """One rotated q (qk=0) or k (qk=1) projection chain for one
                n-block of head pair `pair`."""
                xt_t, cos_sb, sin_sb = pref[(pair, nb)]
                nsl = ts(nb, NB)
                mf = pair + 4 * qk
                qp = psqr.tile([P, NB], F32, tag="qr")
                for kc in range(KT):
                    nc.tensor.matmul(
                        qp,
                        wqk_tiles[pair][:, kc, qk, :],
                        xt_t[:, kc, :],
                        start=(kc == 0),
                        stop=(kc == KT - 1),
                    )
                qa = stg.tile([P, NB], F16, tag="qa")
                nc.vector.tensor_copy(out=qa, in_=qp)
                rp = psqr.tile([P, NB], F32, tag="qr")
                nc.tensor.matmul(rp, r2_sb, qa, start=True, stop=True)
                # t1/add touch only SBUF -> run on the idle gpsimd engine so
                # the Vector FIFO stays short for the attn@v psum evacuation
                # (Vector keeps the two psum readers: qa cast + t2).
                t1 = stg.tile([P, NB], F32, tag="t1")
                nc.gpsimd.tensor_mul(out=t1, in0=qa, in1=cos_sb)
                t2 = stg.tile([P, NB], F32, tag="t2")
                nc.vector.tensor_mul(out=t2, in0=rp, in1=sin_sb)
                nc.gpsimd.tensor_add(out=qk_sb[:, mf, nsl], in0=t1, in1=t2)

            # ---- pass 0 (serial): v projection + head pair 0 ----
            with tc.tile_pool(name="wvtp", bufs=1) as wvtp, \
                 tc.tile_pool(name="psv", bufs=2, space="PSUM") as psv:
                wvt_sb = wvtp.tile([P, KT, 512], BF16)
                xt_first = xtp.tile([P, KT, NB], BF16, tag="xt")
                xt0_r = xt[0]
                # startup order: interleave the pair-0 q-weight chunks with
                # the first x block so the first projection chain can start
                # after ~256KB instead of the full 1.5MB; then cos/sin (used
                # right after the chain), the rotate matrix, the k-weight
                # chunks, and only then the ident (not needed until the
                # weave). wvt streams behind on the gpsimd queue.
                wqk_t0 = wqkp.tile([P, KT, 2, P], BF16, tag="wqk")
                for c4 in range(0, KT, 4):
                    nc.sync.dma_start(
                        out=wqk_t0[:, c4:c4 + 4, 0, :],
                        in_=wqkt[0][:, c4:c4 + 4, :])
                    for kc in range(c4, c4 + 4):
                        nc.sync.dma_start(out=xt_first[:, kc, :],
                                          in_=xt0_r[:, kc, :])
                pref[(0, 0)] = load_xcs(0, xt_first)
                nc.sync.dma_start(out=r2_sb, in_=r2t[:, :])
                for c4 in range(0, KT, 4):
                    nc.sync.dma_start(
                        out=wqk_t0[:, c4:c4 + 4, 1, :],
                        in_=wqkt[4][:, c4:c4 + 4, :])
                wqk_tiles[0] = wqk_t0
                nc.sync.dma_start(out=id_sb, in_=ident[:, :])
                nc.vector.memset(v1_sb[:, :, :, HD:HD + 1], 1.0)
                for kc in range(KT):
                    nc.gpsimd.dma_start(out=wvt_sb[:, kc, :], in_=wvt[:, kc, :])
                for nb in range(NNB):
                    if nb > 0:
                        pref[(0, nb)] = load_xcs(nb)
                    xt_t = pref[(0, nb)][0]
                    qk_half(0, nb, 0)
                    qk_half(0, nb, 1)
                    for t4 in range(NB // P):
                        nt = nb * (NB // P) + t4
                        vp = psv.tile([P, 512], F32, tag="vp")
                        for kc in range(KT):
                            nc.tensor.matmul(
                                vp,
                                xt_t[:, kc, ts(t4, P)],
                                wvt_sb[:, kc, :],
                                start=(kc == 0),
                                stop=(kc == KT - 1),
                            )
                        nc.scalar.copy(
                            out=v1_sb[:, nt, :, 0:HD],
                            in_=vp.rearrange("p (h d) -> p h d", h=NHC),
                        )
                    del pref[(0, nb)]

            for ct in range(4):
                nc.sync.dma_start(out=wot_sb[:, ct, :], in_=wot[:, ct, :])
            load_wqk(1)
            pref[(1, 0)] = load_xcs(0)

            # ---- weave: attention steps (t outer, ib inner) with the q/k
            # projections for pair t+1 and the output projections of
            # completed i-blocks interleaved ----
            with tc.tile_pool(name="expp", bufs=18) as expp, \
                 tc.tile_pool(name="normp", bufs=6) as normp, \
                 tc.tile_pool(name="rdp", bufs=6) as rdp, \
                 tc.tile_pool(name="evp", bufs=3) as evp, \
                 tc.tile_pool(name="psavt", bufs=1, space="PSUM") as psavt, \
                 tc.tile_pool(name="psd", bufs=1, space="PSUM") as psd:

                prev = None          # (t_pair, ib, expq) of the previous step
                pending_tr = []      # [(norm_tile, h2, t_pair, i_slice), ...]
                halves = []          # [(pair, nb, qk), ...] projection work
                pieces = []          # [(it, ob), ...] output-projection work

                def attnv_chain(g):
                    pt, pib, pexpq = prev
                    h2, it2 = divmod(g, 4)
                    h = 2 * pt + h2
                    av = psavt.tile([P, P], F32, tag="av")
                    for jt in range(JT):
                        nc.tensor.matmul(
                            av[:, 0:HD + 1],
                            pexpq[jt // 2][:, jt % 2, h2, ts(it2, P)],
                            v1_sb[:, jt, h, :],
                            start=(jt == 0),
                            stop=(jt == JT - 1),
                        )
                    rd = rdp.tile([P, 1], F32, tag="rd")
                    nc.vector.reciprocal(out=rd, in_=av[:, HD:HD + 1])
                    norm = normp.tile([P, HD], BF16, tag="norm")
                    nc.vector.tensor_scalar_mul(norm, av[:, 0:HD], rd)
                    pending_tr.append((norm, h2, pt, pib, it2))

                def flush_tr():
                    norm, h2, pt, pib, it2 = pending_tr.pop(0)
                    trp = psavt.tile([HD, P], BF16, tag="tr")
                    nc.tensor.transpose(trp, norm, id_sb)
                    nc.vector.tensor_copy(
                        out=att_sb[h2 * HD:(h2 + 1) * HD, pt,
                                   ts(4 * pib + it2, P)],
                        in_=trp,
                    )
                    # the h2=1 flush is the last write this i-tile needs for
                    # pair 3 -> its output projections become available
                    if pt == 3 and h2 == 1:
                        pieces.extend((4 * pib + it2, ob) for ob in range(4))

                proj_pool = [psd]

                def proj_piece():
                    it, ob = pieces.pop(0)
                    op = proj_pool[0].tile([P, 512], F32, tag="op")
                    for ct in range(4):
                        nc.tensor.matmul(
                            op,
                            att_sb[:, ct, ts(it, P)],
                            wot_sb[:, ct, ts(ob, 512)],
                            start=(ct == 0),
                            stop=(ct == 3),
                        )
                    o_sb = evp.tile([P, 512], F32, tag="osb")
                    nc.vector.tensor_copy(out=o_sb, in_=op)
                    nc.gpsimd.dma_start(
                        out=out_part[ts(it, P), ts(ob, 512)], in_=o_sb
                    )

                def run_half():
                    pair, nb, qk = halves.pop(0)
                    if qk == 0:
                        # prefetch the next n-block (or next pass) while this
                        # one computes
                        if nb + 1 < NNB:
                            pref[(pair, nb + 1)] = load_xcs(nb + 1)
                        elif pair < 3:
                            pref[(pair + 1, 0)] = load_xcs(0)
                            load_wqk(pair + 1)
                    qk_half(pair, nb, qk)
                    if qk == 1:
                        del pref[(pair, nb)]

                def work_item(g):
                    if g % 2 == 0 and halves:
                        run_half()
                    elif pieces:
                        proj_piece()
                        if not halves and pieces:
                            proj_piece()

                steps = [(t, ib) for t in range(4) for ib in range(NIB)]
                sc_pools = tc.tile_pool(name="pssc", bufs=1, space="PSUM")
                pssc = sc_pools.__enter__()
                for t, ib in steps:
                    isl = ts(ib, IB)
                    qts = qk_sb[:, t, :]
                    kts = qk_sb[:, 4 + t, :]
                    if t < 3:
                        halves.extend(
                            (t + 1, 2 * ib + x, qk)
                            for x in range(2) for qk in range(2))
                    expq = []
                    for g in range(8):
                        exp_q = expp.tile([P, 2, 2, IB], BF16, tag="exp")
                        expq.append(exp_q)
                        # one 4-bank psum tile per group; the two heads of a
                        # j2 sub-group contract over disjoint 64-partition
                        # bands (row_grp 0 / 64) and run concurrently on the
                        # PE when adjacent, so keep them paired and freed by
                        # a single ACTIVATE per j2.
                        scq = pssc.tile([P, 2, 2, IB], F32, tag="sc")
                        for j2 in range(2):
                            jt = g * 2 + j2
                            nc.tensor.matmul(
                                scq[:, j2, 0, :],
                                kts[0:HD, ts(jt, P)],
                                qts[0:HD, isl],
                                start=True,
                                stop=True,
                            )
                            nc.tensor.matmul(
                                scq[:, j2, 1, :],
                                kts[HD:P, ts(jt, P)],
                                qts[HD:P, isl],
                                start=True,
                                stop=True,
                            )
                        for j2 in range(2):
                            nc.scalar.activation(
                                out=exp_q[:, j2, :, :],
                                in_=scq[:, j2, :, :],
                                func=EXP_FUNC,
                                scale=SCALE,
                            )
                        if pending_tr:
                            flush_tr()
                        if prev is not None:
                            attnv_chain(g)
                        work_item(g)
                    prev = (t, ib, expq)

                sc_pools.__exit__(None, None, None)
                # tail: attn@v of the last step, remaining transposes,
                # remaining output projections (double-buffered psum now that
                # the score pools are closed)
                psd2_cm = tc.tile_pool(name="psd2", bufs=2, space="PSUM")
                proj_pool[0] = psd2_cm.__enter__()
                for g in range(8):
                    if pending_tr:
                        flush_tr()
                    attnv_chain(g)
                    if pieces:
                        proj_piece()
                while pending_tr:
                    flush_tr()
                while pieces:
                    proj_piece()
                psd2_cm.__exit__(None, None, None)


def _rot_matrix():
    r = np.zeros((HD, HD), dtype=np.float32)
    for d in range(32):
        r[d, d + 32] = -1.0
    for d in range(32, HD):
        r[d, d - 32] = 1.0
    r2 = np.zeros((P, P), dtype=np.float32)
    r2[0:HD, 0:HD] = r
    r2[HD:P, HD:P] = r
    return np.ascontiguousarray(r2.T)


def _core_inputs(x, cos_t, sin_t, W_qkv, W_out, core):
    b, g = divmod(core, 4)
    hs = g * NHC * HD  # feature offset of this head group (512 per group)

    xT = np.ascontiguousarray(x[b].T)  # [c, n]
    xt = np.ascontiguousarray(
        xT.reshape(KT, P, NNB, NB).transpose(2, 1, 0, 3)
    ).astype(NPBF16)

    Wq = W_qkv[hs:hs + 512]
    Wk = W_qkv[CDIM + hs:CDIM + hs + 512]
    Wv = W_qkv[2 * CDIM + hs:2 * CDIM + hs + 512]
    WqkT = np.ascontiguousarray(np.concatenate([Wq, Wk], axis=0).T)  # [c, 1024]
    wqkt = np.ascontiguousarray(
        WqkT.reshape(KT, P, MF, P).transpose(2, 1, 0, 3)
    ).astype(NPBF16)
    WvT = np.ascontiguousarray(Wv.T)  # [c, 512]
    wvt = np.ascontiguousarray(
        WvT.reshape(KT, P, 512).transpose(1, 0, 2)).astype(NPBF16)
    WoT = np.ascontiguousarray(W_out[:, hs:hs + 512].T)  # [c-slice 512, o 2048]
    wot = np.ascontiguousarray(
        WoT.reshape(4, P, CDIM).transpose(1, 0, 2)).astype(NPBF16)

    return {
        "xt": xt,
        "wqkt": wqkt,
        "wvt": wvt,
        "wot": wot,
        "cos2t": cos_t,
        "sin2t": sin_t,
        "r2t": _ROT,
        "ident": _IDENT,
    }


_ROT = _rot_matrix().astype(np.float16)
_IDENT = np.eye(P, dtype=np.float32).astype(NPBF16)


def kernel(x, freqs, W_qkv, W_out):
    global _CACHED_NC
    x = np.asarray(x, dtype=np.float32)
    freqs = np.asarray(freqs, dtype=np.float32)
    W_qkv = np.asarray(W_qkv, dtype=np.float32)
    W_out = np.asarray(W_out, dtype=np.float32)

    if _CACHED_NC is None:
        _CACHED_NC = _build_nc()
    nc = _CACHED_NC

    cos_t = np.ascontiguousarray(
        np.tile(np.cos(freqs.T), (2, 1))).astype(NPBF16)  # [128, n]
    sin_t = np.ascontiguousarray(
        np.tile(np.sin(freqs.T), (2, 1))).astype(NPBF16)

    in_maps = [
        _core_inputs(x, cos_t, sin_t, W_qkv, W_out, core) for core in range(8)
    ]
    trace = os.environ.get("ATT_TRACE") == "1"
    res = run_bass_kernel_spmd(nc, in_maps, core_ids=list(range(8)), trace=trace)
    if trace and res.exec_time_ns is not None:
        print(f"HW exec time: {res.exec_time_ns} ns")

    out = np.empty((2, NSEQ, CDIM), dtype=np.float32)
    for b in range(2):
        acc = np.zeros((NSEQ, CDIM), dtype=np.float64)
        for g in range(4):
            acc += res.results[4 * b + g]["out_part"]
        out[b] = acc.astype(np.float32)
    return out



# revision 23
# speedup vs baseline: 1.1561x; 1.1561x over previous
"""Trainium2 Bass kernel for the fused attention module.

8-core sharding: data-parallel over batch (B=2) x tensor-parallel over head
groups (32 heads -> 4 groups of 8). Core c handles batch c//4, head group c%4.
Each core computes QKV projection (its head slice), RoPE, full non-causal
attention for its 8 heads, and a partial output projection against its
W_out column slice; the host sums the 4 partials per batch.

v2 design (vs baseline):
- all matmul operands in bf16 (same PE rate as f32r at N>=256 per the cost
  model, half the DMA/SBUF); rotary math stays f32, one bf16 rounding on the
  stored q/k
- q/k stay resident in SBUF ([128c, 8mf, 2048n] bf16) -- no DRAM spill
- attn@v computed transposed: out[i, d] via lhsT=exp[j, i-tile], rhs=v[j, 65]
  (M=128 instead of 65 -> half the PE time); the 65th v column of ones gives
  the softmax denominator; normalize = per-partition reciprocal broadcast;
  PE-transpose (identity matmul) back to [d, i] for the output projection
- phase CD is software-pipelined ("weave"): per head-pair step, the 8
  score-groups (PE) + exps (Act) are interleaved with the previous pair's
  attn@v chains and the previous i-block's output-projection pieces so the
  Activation engine (CD bottleneck ~255us of exp) never starves.

v3 (vs v2):
- scores exploit PE row tiling: the two heads of a pair contract over
  disjoint 64-partition bands (row_grp h0/h64), and the hardware runs
  such matmuls CONCURRENTLY when adjacent in the engine stream (~3ns
  stagger). v2 separated them because sc0/sc1 had separate psum pools
  freed by two sequential ACTIVATEs; v3 puts all 4 score matmuls of a
  group in ONE 4-bank psum tile consumed by ONE ACTIVATE, so all four
  become schedule-ready together and pair up h0/h64 back-to-back.
- rotary rotate-half matmul in bf16 (was f32r at ~2.7x the cycles); the
  permutation matrix is exact in bf16, input rounding is ~0.4%.
- startup: first-needed DMAs (wqk pair 0 chunked, x block 0) issue ahead
  of the ident/rot-matrix loads so the first projection matmul starts
  sooner.

Orientation notes (PE computes out = lhsT.T @ rhs, contraction on partitions):
- qT/kT produced as [f, n] (lhsT = W slice pre-transposed on host, rhs = xT)
- v produced as [n, f] (lhsT = xT tile, rhs = WvT)
- scoresT[j, i] per head (lhsT = kT j-tile, rhs = qT i-block)
- RoPE rotate_half is a partition shift via a constant permutation matmul
"""

import os
import sys

sys.path.insert(0, "/opt/trn_rl_repo")

import numpy as np
import ml_dtypes

import concourse.bass as bass  # noqa: F401
import concourse.mybir as mybir
import concourse.tile as tile
from concourse import bacc
from concourse.bass import ts
from concourse.bass_utils import run_bass_kernel_spmd

F32 = mybir.dt.float32
F32R = mybir.dt.float32r
F16 = mybir.dt.float16
BF16 = mybir.dt.bfloat16
NPBF16 = ml_dtypes.bfloat16

P = 128
NSEQ = 2048          # sequence length
CDIM = 2048          # model dim
HD = 64              # head dim
NHC = 8              # heads per core
KT = CDIM // P       # 16 contraction tiles
NB = 256             # n-block in the fused projection phase
NNB = NSEQ // NB     # 8
IB = 256             # i-block in attention
NIB = NSEQ // IB     # 8
JT = NSEQ // P       # 16 j-tiles
MF = 8               # q/k feature tiles (0-3 q, 4-7 k)
EXP_FUNC = mybir.ActivationFunctionType.Exp
SCALE = 1.0 / 8.0    # 1/sqrt(HD)

_CACHED_NC = None


def _build_nc():
    nc = bacc.Bacc(None)

    xt = nc.declare_dram_parameter("xt", [NNB, P, KT, NB], BF16, isOutput=False)
    wqkt = nc.declare_dram_parameter("wqkt", [MF, P, KT, P], BF16, isOutput=False)
    wvt = nc.declare_dram_parameter("wvt", [P, KT, 512], BF16, isOutput=False)
    wot = nc.declare_dram_parameter("wot", [P, 4, CDIM], BF16, isOutput=False)
    cos2t = nc.declare_dram_parameter("cos2t", [P, NSEQ], BF16, isOutput=False)
    sin2t = nc.declare_dram_parameter("sin2t", [P, NSEQ], BF16, isOutput=False)
    r2t = nc.declare_dram_parameter("r2t", [P, P], F16, isOutput=False)
    ident = nc.declare_dram_parameter("ident", [P, P], BF16, isOutput=False)
    out_part = nc.declare_dram_parameter("out_part", [NSEQ, CDIM], F32, isOutput=True)

    n_repeat = int(os.environ.get("ATT_REPEAT", "1"))

    with tile.TileContext(nc) as tc, nc.allow_low_precision("bf16 matmul kernel"):
        for _rep in range(n_repeat):
            _kernel_body(nc, tc, xt, wqkt, wvt, wot, cos2t, sin2t, r2t,
                         ident, out_part)

    nc.compile()
    return nc


def _kernel_body(nc, tc, xt, wqkt, wvt, wot, cos2t, sin2t, r2t, ident,
                 out_part):
    with tc.tile_pool(name="persist", bufs=1) as persist:
        qk_sb = persist.tile([P, MF, NSEQ], BF16)
        v1_sb = persist.tile([P, JT, NHC, HD + 1], BF16)
        att_sb = persist.tile([P, 4, NSEQ], BF16)
        wot_sb = persist.tile([P, 4, CDIM], BF16)
        id_sb = persist.tile([P, P], BF16)
        r2_sb = persist.tile([P, P], F16)

        # pools that span pass 0 AND the weave (q/k projection streaming)
        with tc.tile_pool(name="wqkp", bufs=2) as wqkp, \
             tc.tile_pool(name="xtp", bufs=2) as xtp, \
             tc.tile_pool(name="csp", bufs=2) as csp, \
             tc.tile_pool(name="stg", bufs=3) as stg, \
             tc.tile_pool(name="psqr", bufs=1, space="PSUM") as psqr:

            wqk_tiles = {}

            def load_wqk(pair):
                wqk_t = wqkp.tile([P, KT, 2, P], BF16, tag="wqk")
                nc.sync.dma_start(out=wqk_t[:, :, 0, :], in_=wqkt[pair])
                nc.sync.dma_start(out=wqk_t[:, :, 1, :], in_=wqkt[4 + pair])
                wqk_tiles[pair] = wqk_t

            pref = {}

            def load_xcs(nb, first=None):
                if first is not None:
                    xt_t = first
                else:
                    xt_t = xtp.tile([P, KT, NB], BF16, tag="xt")
                    nc.sync.dma_start(out=xt_t, in_=xt[nb])
                cos_sb = csp.tile([P, NB], BF16, tag="cos")
                sin_sb = csp.tile([P, NB], BF16, tag="sin")
                nc.sync.dma_start(out=cos_sb, in_=cos2t[:, ts(nb, NB)])
                nc.sync.dma_start(out=sin_sb, in_=sin2t[:, ts(nb, NB)])
                return xt_t, cos_sb, sin_sb

            def qk_half(pair, nb, qk):
                """One rotated q (qk=0) or k (qk=1) projection chain for one
                n-block of head pair `pair`."""
                xt_t, cos_sb, sin_sb = pref[(pair, nb)]
                nsl = ts(nb, NB)
                mf = pair + 4 * qk
                qp = psqr.tile([P, NB], F32, tag="qr")
                for kc in range(KT):
                    nc.tensor.matmul(
                        qp,
                        wqk_tiles[pair][:, kc, qk, :],
                        xt_t[:, kc, :],
                        start=(kc == 0),
                        stop=(kc == KT - 1),
                    )
                qa = stg.tile([P, NB], F16, tag="qa")
                nc.vector.tensor_copy(out=qa, in_=qp)
                rp = psqr.tile([P, NB], F32, tag="qr")
                nc.tensor.matmul(rp, r2_sb, qa, start=True, stop=True)
                # t1/add touch only SBUF -> run on the idle gpsimd engine so
                # the Vector FIFO stays short for the attn@v psum evacuation
                # (Vector keeps the two psum readers: qa cast + t2).
                t1 = stg.tile([P, NB], F32, tag="t1")
                nc.gpsimd.tensor_mul(out=t1, in0=qa, in1=cos_sb)
                t2 = stg.tile([P, NB], F32, tag="t2")
                nc.vector.tensor_mul(out=t2, in0=rp, in1=sin_sb)
                nc.gpsimd.tensor_add(out=qk_sb[:, mf, nsl], in0=t1, in1=t2)

            # ---- pass 0 (serial): v projection + head pair 0 ----
            with tc.tile_pool(name="wvtp", bufs=1) as wvtp, \
                 tc.tile_pool(name="psv", bufs=2, space="PSUM") as psv:
                wvt_sb = wvtp.tile([P, KT, 512], BF16)
                xt_first = xtp.tile([P, KT, NB], BF16, tag="xt")
                xt0_r = xt[0]
                # startup order: interleave the pair-0 q-weight chunks with
                # the first x block so the first projection chain can start
                # after ~256KB instead of the full 1.5MB; then cos/sin (used
                # right after the chain), the rotate matrix, the k-weight
                # chunks, and only then the ident (not needed until the
                # weave). wvt streams behind on the gpsimd queue.
                wqk_t0 = wqkp.tile([P, KT, 2, P], BF16, tag="wqk")
                for c4 in range(0, KT, 4):
                    nc.sync.dma_start(
                        out=wqk_t0[:, c4:c4 + 4, 0, :],
                        in_=wqkt[0][:, c4:c4 + 4, :])
                    for kc in range(c4, c4 + 4):
                        nc.sync.dma_start(out=xt_first[:, kc, :],
                                          in_=xt0_r[:, kc, :])
                pref[(0, 0)] = load_xcs(0, xt_first)
                nc.sync.dma_start(out=r2_sb, in_=r2t[:, :])
                for c4 in range(0, KT, 4):
                    nc.sync.dma_start(
                        out=wqk_t0[:, c4:c4 + 4, 1, :],
                        in_=wqkt[4][:, c4:c4 + 4, :])
                wqk_tiles[0] = wqk_t0
                nc.sync.dma_start(out=id_sb, in_=ident[:, :])
                nc.vector.memset(v1_sb[:, :, :, HD:HD + 1], 1.0)
                for kc in range(KT):
                    nc.gpsimd.dma_start(out=wvt_sb[:, kc, :], in_=wvt[:, kc, :])
                for nb in range(NNB):
                    if nb > 0:
                        pref[(0, nb)] = load_xcs(nb)
                    xt_t = pref[(0, nb)][0]
                    qk_half(0, nb, 0)
                    qk_half(0, nb, 1)
                    for t4 in range(NB // P):
                        nt = nb * (NB // P) + t4
                        vp = psv.tile([P, 512], F32, tag="vp")
                        for kc in range(KT):
                            nc.tensor.matmul(
                                vp,
                                xt_t[:, kc, ts(t4, P)],
                                wvt_sb[:, kc, :],
                                start=(kc == 0),
                                stop=(kc == KT - 1),
                            )
                        nc.scalar.copy(
                            out=v1_sb[:, nt, :, 0:HD],
                            in_=vp.rearrange("p (h d) -> p h d", h=NHC),
                        )
                    del pref[(0, nb)]

            for ct in range(4):
                nc.sync.dma_start(out=wot_sb[:, ct, :], in_=wot[:, ct, :])
            load_wqk(1)
            pref[(1, 0)] = load_xcs(0)

            # ---- weave: attention steps (t outer, ib inner) with the q/k
            # projections for pair t+1 and the output projections of
            # completed i-blocks interleaved ----
            with tc.tile_pool(name="expp", bufs=18) as expp, \
                 tc.tile_pool(name="normp", bufs=6) as normp, \
                 tc.tile_pool(name="rdp", bufs=6) as rdp, \
                 tc.tile_pool(name="evp", bufs=3) as evp, \
                 tc.tile_pool(name="psavt", bufs=1, space="PSUM") as psavt, \
                 tc.tile_pool(name="psd", bufs=1, space="PSUM") as psd:

                prev = None          # (t_pair, ib, expq) of the previous step
                pending_tr = []      # [(norm_tile, h2, t_pair, i_slice), ...]
                halves = []          # [(pair, nb, qk), ...] projection work
                pieces = []          # [(it, ob), ...] output-projection work

                def attnv_chain(c):
                    pt, pib, pexpq = prev
                    h2, it2 = divmod(c, 2)
                    h = 2 * pt + h2
                    av = psavt.tile([P, P], F32, tag="av")
                    for jt in range(JT):
                        nc.tensor.matmul(
                            av[:, 0:HD + 1],
                            pexpq[jt // 2][:, h2, jt % 2, ts(it2, P)],
                            v1_sb[:, jt, h, :],
                            start=(jt == 0),
                            stop=(jt == JT - 1),
                        )
                    rd = rdp.tile([P, 1], F32, tag="rd")
                    nc.vector.reciprocal(out=rd, in_=av[:, HD:HD + 1])
                    norm = normp.tile([P, HD], BF16, tag="norm")
                    nc.vector.tensor_scalar_mul(norm, av[:, 0:HD], rd)
                    pending_tr.append((norm, h2, pt, pib, it2))

                def flush_tr():
                    norm, h2, pt, pib, it2 = pending_tr.pop(0)
                    trp = psavt.tile([HD, P], BF16, tag="tr")
                    nc.tensor.transpose(trp, norm, id_sb)
                    nc.vector.tensor_copy(
                        out=att_sb[h2 * HD:(h2 + 1) * HD, pt,
                                   ts(2 * pib + it2, P)],
                        in_=trp,
                    )
                    # the h2=1 flush is the last write this i-tile needs for
                    # pair 3 -> its output projections become available
                    if pt == 3 and h2 == 1:
                        pieces.extend((2 * pib + it2, ob) for ob in range(4))

                proj_pool = [psd]

                def proj_piece():
                    it, ob = pieces.pop(0)
                    op = proj_pool[0].tile([P, 512], F32, tag="op")
                    for ct in range(4):
                        nc.tensor.matmul(
                            op,
                            att_sb[:, ct, ts(it, P)],
                            wot_sb[:, ct, ts(ob, 512)],
                            start=(ct == 0),
                            stop=(ct == 3),
                        )
                    o_sb = evp.tile([P, 512], F32, tag="osb")
                    nc.vector.tensor_copy(out=o_sb, in_=op)
                    nc.gpsimd.dma_start(
                        out=out_part[ts(it, P), ts(ob, 512)], in_=o_sb
                    )

                def run_half():
                    pair, nb, qk = halves.pop(0)
                    if qk == 0:
                        # prefetch the next n-block (or next pass) while this
                        # one computes
                        if nb + 1 < NNB:
                            pref[(pair, nb + 1)] = load_xcs(nb + 1)
                        elif pair < 3:
                            pref[(pair + 1, 0)] = load_xcs(0)
                            load_wqk(pair + 1)
                    qk_half(pair, nb, qk)
                    if qk == 1:
                        del pref[(pair, nb)]

                def work_item(slot):
                    if slot % 2 == 0 and halves:
                        run_half()
                    elif pieces:
                        proj_piece()
                        if not halves and pieces:
                            proj_piece()
                    elif halves:
                        run_half()

                steps = [(t, ib) for t in range(4) for ib in range(NIB)]
                sc_pools = tc.tile_pool(name="pssc", bufs=2, space="PSUM")
                pssc = sc_pools.__enter__()
                for t, ib in steps:
                    isl = ts(ib, IB)
                    qts = qk_sb[:, t, :]
                    kts = qk_sb[:, 4 + t, :]
                    if t < 3:
                        halves.extend((t + 1, ib, qk) for qk in range(2))
                    expq = []
                    for g in range(8):
                        exp_q = expp.tile([P, 2, 2, IB], BF16, tag="exp")
                        expq.append(exp_q)
                        # one 2-bank psum tile per group, double-buffered so
                        # group g+1's matmuls only wait on the ACTIVATE of
                        # g-1. The two heads of a j2 sub-group contract over
                        # disjoint 64-partition bands (row_grp 0 / 64) and
                        # run concurrently on the PE when adjacent in the
                        # stream, so keep them paired. Layout is [h2, j2, i]
                        # (h2 outermost, one full 2KB bank per head): two
                        # CONCURRENT row-band matmuls must not write the same
                        # psum bank -- with [j2, h2, i] at IB=256 the h0/h64
                        # pair lands in two halves of one bank and the device
                        # faults.
                        scq = pssc.tile([P, 2, 2, IB], F32, tag="sc")
                        for j2 in range(2):
                            jt = g * 2 + j2
                            nc.tensor.matmul(
                                scq[:, 0, j2, :],
                                kts[0:HD, ts(jt, P)],
                                qts[0:HD, isl],
                                start=True,
                                stop=True,
                            )
                            nc.tensor.matmul(
                                scq[:, 1, j2, :],
                                kts[HD:P, ts(jt, P)],
                                qts[HD:P, isl],
                                start=True,
                                stop=True,
                            )
                        for j2 in range(2):
                            nc.scalar.activation(
                                out=exp_q[:, :, j2, :],
                                in_=scq[:, :, j2, :],
                                func=EXP_FUNC,
                                scale=SCALE,
                            )
                        if pending_tr:
                            flush_tr()
                        if prev is not None and g % 2 == 0:
                            attnv_chain(g // 2)
                        if g % 2 == 1:
                            work_item(g // 2)
                    prev = (t, ib, expq)

                sc_pools.__exit__(None, None, None)
                # tail: attn@v of the last step, remaining transposes,
                # remaining output projections (double-buffered psum now that
                # the score pool is closed)
                psd2_cm = tc.tile_pool(name="psd2", bufs=2, space="PSUM")
                proj_pool[0] = psd2_cm.__enter__()
                for c in range(4):
                    if pending_tr:
                        flush_tr()
                    attnv_chain(c)
                    if pieces:
                        proj_piece()
                    if pieces:
                        proj_piece()
                while pending_tr:
                    flush_tr()
                while pieces:
                    proj_piece()
                psd2_cm.__exit__(None, None, None)


def _rot_matrix():
    r = np.zeros((HD, HD), dtype=np.float32)
    for d in range(32):
        r[d, d + 32] = -1.0
    for d in range(32, HD):
        r[d, d - 32] = 1.0
    r2 = np.zeros((P, P), dtype=np.float32)
    r2[0:HD, 0:HD] = r
    r2[HD:P, HD:P] = r
    return np.ascontiguousarray(r2.T)


def _core_inputs(x, cos_t, sin_t, W_qkv, W_out, core):
    b, g = divmod(core, 4)
    hs = g * NHC * HD  # feature offset of this head group (512 per group)

    xT = np.ascontiguousarray(x[b].T)  # [c, n]
    xt = np.ascontiguousarray(
        xT.reshape(KT, P, NNB, NB).transpose(2, 1, 0, 3)
    ).astype(NPBF16)

    Wq = W_qkv[hs:hs + 512]
    Wk = W_qkv[CDIM + hs:CDIM + hs + 512]
    Wv = W_qkv[2 * CDIM + hs:2 * CDIM + hs + 512]
    WqkT = np.ascontiguousarray(np.concatenate([Wq, Wk], axis=0).T)  # [c, 1024]
    wqkt = np.ascontiguousarray(
        WqkT.reshape(KT, P, MF, P).transpose(2, 1, 0, 3)
    ).astype(NPBF16)
    WvT = np.ascontiguousarray(Wv.T)  # [c, 512]
    wvt = np.ascontiguousarray(
        WvT.reshape(KT, P, 512).transpose(1, 0, 2)).astype(NPBF16)
    WoT = np.ascontiguousarray(W_out[:, hs:hs + 512].T)  # [c-slice 512, o 2048]
    wot = np.ascontiguousarray(
        WoT.reshape(4, P, CDIM).transpose(1, 0, 2)).astype(NPBF16)

    return {
        "xt": xt,
        "wqkt": wqkt,
        "wvt": wvt,
        "wot": wot,
        "cos2t": cos_t,
        "sin2t": sin_t,
        "r2t": _ROT,
        "ident": _IDENT,
    }


_ROT = _rot_matrix().astype(np.float16)
_IDENT = np.eye(P, dtype=np.float32).astype(NPBF16)


def kernel(x, freqs, W_qkv, W_out):
    global _CACHED_NC
    x = np.asarray(x, dtype=np.float32)
    freqs = np.asarray(freqs, dtype=np.float32)
    W_qkv = np.asarray(W_qkv, dtype=np.float32)
    W_out = np.asarray(W_out, dtype=np.float32)

    if _CACHED_NC is None:
        _CACHED_NC = _build_nc()
    nc = _CACHED_NC

    cos_t = np.ascontiguousarray(
        np.tile(np.cos(freqs.T), (2, 1))).astype(NPBF16)  # [128, n]
    sin_t = np.ascontiguousarray(
        np.tile(np.sin(freqs.T), (2, 1))).astype(NPBF16)

    in_maps = [
        _core_inputs(x, cos_t, sin_t, W_qkv, W_out, core) for core in range(8)
    ]
    trace = os.environ.get("ATT_TRACE") == "1"
    res = run_bass_kernel_spmd(nc, in_maps, core_ids=list(range(8)), trace=trace)
    if trace and res.exec_time_ns is not None:
        print(f"HW exec time: {res.exec_time_ns} ns")

    out = np.empty((2, NSEQ, CDIM), dtype=np.float32)
    for b in range(2):
        acc = np.zeros((NSEQ, CDIM), dtype=np.float64)
        for g in range(4):
            acc += res.results[4 * b + g]["out_part"]
        out[b] = acc.astype(np.float32)
    return out



# revision 29
# speedup vs baseline: 1.1999x; 1.0379x over previous
"""Trainium2 Bass kernel for the fused attention module.

8-core sharding: data-parallel over batch (B=2) x tensor-parallel over head
groups (32 heads -> 4 groups of 8). Core c handles batch c//4, head group c%4.
Each core computes QKV projection (its head slice), RoPE, full non-causal
attention for its 8 heads, and a partial output projection against its
W_out column slice; the host sums the 4 partials per batch.

v2 design (vs baseline):
- all matmul operands in bf16 (same PE rate as f32r at N>=256 per the cost
  model, half the DMA/SBUF); rotary math stays f32, one bf16 rounding on the
  stored q/k
- q/k stay resident in SBUF ([128c, 8mf, 2048n] bf16) -- no DRAM spill
- attn@v computed transposed: out[i, d] via lhsT=exp[j, i-tile], rhs=v[j, 65]
  (M=128 instead of 65 -> half the PE time); the 65th v column of ones gives
  the softmax denominator; normalize = per-partition reciprocal broadcast;
  PE-transpose (identity matmul) back to [d, i] for the output projection
- phase CD is software-pipelined ("weave"): per head-pair step, the 8
  score-groups (PE) + exps (Act) are interleaved with the previous pair's
  attn@v chains and the previous i-block's output-projection pieces so the
  Activation engine (CD bottleneck ~255us of exp) never starves.

v3 (vs v2):
- scores exploit PE row tiling: the two heads of a pair contract over
  disjoint 64-partition bands (row_grp h0/h64), and the hardware runs
  such matmuls CONCURRENTLY when adjacent in the engine stream (~3ns
  stagger). v2 separated them because sc0/sc1 had separate psum pools
  freed by two sequential ACTIVATEs; v3 puts all 4 score matmuls of a
  group in ONE 4-bank psum tile consumed by ONE ACTIVATE, so all four
  become schedule-ready together and pair up h0/h64 back-to-back.
- rotary rotate-half matmul in bf16 (was f32r at ~2.7x the cycles); the
  permutation matrix is exact in bf16, input rounding is ~0.4%.
- startup: first-needed DMAs (wqk pair 0 chunked, x block 0) issue ahead
  of the ident/rot-matrix loads so the first projection matmul starts
  sooner.

Orientation notes (PE computes out = lhsT.T @ rhs, contraction on partitions):
- qT/kT produced as [f, n] (lhsT = W slice pre-transposed on host, rhs = xT)
- v produced as [n, f] (lhsT = xT tile, rhs = WvT)
- scoresT[j, i] per head (lhsT = kT j-tile, rhs = qT i-block)
- RoPE rotate_half is a partition shift via a constant permutation matmul
"""

import os
import sys

sys.path.insert(0, "/opt/trn_rl_repo")

import numpy as np
import ml_dtypes

import concourse.bass as bass  # noqa: F401
import concourse.mybir as mybir
import concourse.tile as tile
from concourse import bacc
from concourse.bass import ts
from concourse.bass_utils import run_bass_kernel_spmd

F32 = mybir.dt.float32
F32R = mybir.dt.float32r
F16 = mybir.dt.float16
BF16 = mybir.dt.bfloat16
NPBF16 = ml_dtypes.bfloat16

P = 128
NSEQ = 2048          # sequence length
CDIM = 2048          # model dim
HD = 64              # head dim
NHC = 8              # heads per core
KT = CDIM // P       # 16 contraction tiles
NB = 512             # n-block in the fused projection phase
NNB = NSEQ // NB     # 4
IB = 256             # i-block in attention
NIB = NSEQ // IB     # 8
JT = NSEQ // P       # 16 j-tiles
MF = 8               # q/k feature tiles (0-3 q, 4-7 k)
EXP_FUNC = mybir.ActivationFunctionType.Exp
SCALE = 1.0 / 8.0    # 1/sqrt(HD)

_CACHED_NC = None


def _build_nc():
    nc = bacc.Bacc(None)

    xt = nc.declare_dram_parameter("xt", [NNB, P, KT, NB], BF16, isOutput=False)
    wqkt = nc.declare_dram_parameter("wqkt", [MF, P, KT, P], BF16, isOutput=False)
    wvt = nc.declare_dram_parameter("wvt", [P, KT, 512], BF16, isOutput=False)
    wot = nc.declare_dram_parameter("wot", [P, 4, CDIM], BF16, isOutput=False)
    cos2t = nc.declare_dram_parameter("cos2t", [P, NSEQ], BF16, isOutput=False)
    sin2t = nc.declare_dram_parameter("sin2t", [P, NSEQ], BF16, isOutput=False)
    r2t = nc.declare_dram_parameter("r2t", [P, P], F16, isOutput=False)
    ident = nc.declare_dram_parameter("ident", [P, P], BF16, isOutput=False)
    out_part = nc.declare_dram_parameter("out_part", [NSEQ, CDIM], F32, isOutput=True)

    n_repeat = int(os.environ.get("ATT_REPEAT", "1"))

    with tile.TileContext(nc) as tc, nc.allow_low_precision("bf16 matmul kernel"):
        for _rep in range(n_repeat):
            _kernel_body(nc, tc, xt, wqkt, wvt, wot, cos2t, sin2t, r2t,
                         ident, out_part)

    nc.compile()
    return nc


def _kernel_body(nc, tc, xt, wqkt, wvt, wot, cos2t, sin2t, r2t, ident,
                 out_part):
    with tc.tile_pool(name="persist", bufs=1) as persist:
        qk_sb = persist.tile([P, MF, NSEQ], BF16)
        v1_sb = persist.tile([P, JT, NHC, HD + 1], BF16)
        att_sb = persist.tile([P, 4, NSEQ], BF16)
        wot_sb = persist.tile([P, 4, CDIM], BF16)
        id_sb = persist.tile([P, P], BF16)
        r2_sb = persist.tile([P, P], F16)

        # pools that span pass 0 AND the weave (q/k projection streaming)
        with tc.tile_pool(name="wqkp", bufs=2) as wqkp, \
             tc.tile_pool(name="xtp", bufs=2) as xtp, \
             tc.tile_pool(name="csp", bufs=2) as csp, \
             tc.tile_pool(name="stg", bufs=3) as stg, \
             tc.tile_pool(name="psqr", bufs=1, space="PSUM") as psqr:

            wqk_tiles = {}

            def load_wqk(pair):
                wqk_t = wqkp.tile([P, KT, 2, P], BF16, tag="wqk")
                nc.sync.dma_start(out=wqk_t[:, :, 0, :], in_=wqkt[pair])
                nc.sync.dma_start(out=wqk_t[:, :, 1, :], in_=wqkt[4 + pair])
                wqk_tiles[pair] = wqk_t

            pref = {}

            def load_xcs(nb, first=None):
                if first is not None:
                    xt_t = first
                else:
                    xt_t = xtp.tile([P, KT, NB], BF16, tag="xt")
                    nc.sync.dma_start(out=xt_t, in_=xt[nb])
                cos_sb = csp.tile([P, NB], BF16, tag="cos")
                sin_sb = csp.tile([P, NB], BF16, tag="sin")
                nc.sync.dma_start(out=cos_sb, in_=cos2t[:, ts(nb, NB)])
                nc.sync.dma_start(out=sin_sb, in_=sin2t[:, ts(nb, NB)])
                return xt_t, cos_sb, sin_sb

            def qk_half(pair, nb, qk):
                """One rotated q (qk=0) or k (qk=1) projection chain for one
                n-block of head pair `pair`."""
                xt_t, cos_sb, sin_sb = pref[(pair, nb)]
                nsl = ts(nb, NB)
                mf = pair + 4 * qk
                qp = psqr.tile([P, NB], F32, tag="qr")
                for kc in range(KT):
                    nc.tensor.matmul(
                        qp,
                        wqk_tiles[pair][:, kc, qk, :],
                        xt_t[:, kc, :],
                        start=(kc == 0),
                        stop=(kc == KT - 1),
                    )
                qa = stg.tile([P, NB], F16, tag="qa")
                nc.vector.tensor_copy(out=qa, in_=qp)
                rp = psqr.tile([P, NB], F32, tag="qr")
                nc.tensor.matmul(rp, r2_sb, qa, start=True, stop=True)
                # t1/add touch only SBUF -> run on the idle gpsimd engine so
                # the Vector FIFO stays short for the attn@v psum evacuation
                # (Vector keeps the two psum readers: qa cast + t2).
                t1 = stg.tile([P, NB], F32, tag="t1")
                nc.gpsimd.tensor_mul(out=t1, in0=qa, in1=cos_sb)
                t2 = stg.tile([P, NB], F32, tag="t2")
                nc.vector.tensor_mul(out=t2, in0=rp, in1=sin_sb)
                nc.gpsimd.tensor_add(out=qk_sb[:, mf, nsl], in0=t1, in1=t2)

            # ---- pass 0 (serial): v projection + head pair 0 ----
            with tc.tile_pool(name="wvtp", bufs=1) as wvtp, \
                 tc.tile_pool(name="psv", bufs=2, space="PSUM") as psv:
                wvt_sb = wvtp.tile([P, KT, 512], BF16)
                xt_first = xtp.tile([P, KT, NB], BF16, tag="xt")
                xt0_r = xt[0]
                # startup order: interleave the pair-0 q-weight chunks with
                # the first x block so the first projection chain can start
                # after ~256KB instead of the full 1.5MB; then cos/sin (used
                # right after the chain), the rotate matrix, the k-weight
                # chunks, and only then the ident (not needed until the
                # weave). wvt streams behind on the gpsimd queue.
                wqk_t0 = wqkp.tile([P, KT, 2, P], BF16, tag="wqk")
                for c4 in range(0, KT, 4):
                    nc.sync.dma_start(
                        out=wqk_t0[:, c4:c4 + 4, 0, :],
                        in_=wqkt[0][:, c4:c4 + 4, :])
                    for kc in range(c4, c4 + 4):
                        nc.sync.dma_start(out=xt_first[:, kc, :],
                                          in_=xt0_r[:, kc, :])
                    if c4 == 0:
                        pref[(0, 0)] = load_xcs(0, xt_first)
                        nc.sync.dma_start(out=r2_sb, in_=r2t[:, :])
                for c4 in range(0, KT, 4):
                    nc.sync.dma_start(
                        out=wqk_t0[:, c4:c4 + 4, 1, :],
                        in_=wqkt[4][:, c4:c4 + 4, :])
                wqk_tiles[0] = wqk_t0
                nc.sync.dma_start(out=id_sb, in_=ident[:, :])
                nc.vector.memset(v1_sb[:, :, :, HD:HD + 1], 1.0)
                for kc in range(KT):
                    nc.gpsimd.dma_start(out=wvt_sb[:, kc, :], in_=wvt[:, kc, :])
                for nb in range(NNB):
                    if nb > 0:
                        pref[(0, nb)] = load_xcs(nb)
                    xt_t = pref[(0, nb)][0]
                    qk_half(0, nb, 0)
                    qk_half(0, nb, 1)
                    for t4 in range(NB // P):
                        nt = nb * (NB // P) + t4
                        vp = psv.tile([P, 512], F32, tag="vp")
                        for kc in range(KT):
                            nc.tensor.matmul(
                                vp,
                                xt_t[:, kc, ts(t4, P)],
                                wvt_sb[:, kc, :],
                                start=(kc == 0),
                                stop=(kc == KT - 1),
                            )
                        nc.scalar.copy(
                            out=v1_sb[:, nt, :, 0:HD],
                            in_=vp.rearrange("p (h d) -> p h d", h=NHC),
                        )
                    del pref[(0, nb)]

            for ct in range(4):
                nc.sync.dma_start(out=wot_sb[:, ct, :], in_=wot[:, ct, :])
            load_wqk(1)
            pref[(1, 0)] = load_xcs(0)

            # ---- weave: attention steps (t outer, ib inner) with the q/k
            # projections for pair t+1 and the output projections of
            # completed i-blocks interleaved ----
            with tc.tile_pool(name="expp", bufs=18) as expp, \
                 tc.tile_pool(name="normp", bufs=6) as normp, \
                 tc.tile_pool(name="rdp", bufs=6) as rdp, \
                 tc.tile_pool(name="evp", bufs=3) as evp, \
                 tc.tile_pool(name="psavt", bufs=1, space="PSUM") as psavt, \
                 tc.tile_pool(name="psav2", bufs=2, space="PSUM") as psav2:

                prev = None          # (t_pair, ib, expq) of the previous step
                pending_tr = []      # [(norm_tile, h2, t_pair, i_slice), ...]
                halves = []          # [(pair, nb, qk), ...] projection work
                pieces = []          # [(it, ob), ...] output-projection work

                def attnv_chain(c):
                    pt, pib, pexpq = prev
                    h2, it2 = divmod(c, 2)
                    h = 2 * pt + h2
                    # double-buffered: the next chain's accumulation does not
                    # wait for this chain's Vector evacuation (recip + mul)
                    av = psav2.tile([P, P], F32, tag="av")
                    for jt in range(JT):
                        nc.tensor.matmul(
                            av[:, 0:HD + 1],
                            pexpq[jt // 2][:, h2, jt % 2, ts(it2, P)],
                            v1_sb[:, jt, h, :],
                            start=(jt == 0),
                            stop=(jt == JT - 1),
                        )
                    rd = rdp.tile([P, 1], F32, tag="rd")
                    nc.vector.reciprocal(out=rd, in_=av[:, HD:HD + 1])
                    norm = normp.tile([P, HD], BF16, tag="norm")
                    nc.vector.tensor_scalar_mul(norm, av[:, 0:HD], rd)
                    pending_tr.append((norm, h2, pt, pib, it2))

                def flush_tr():
                    norm, h2, pt, pib, it2 = pending_tr.pop(0)
                    trp = psavt.tile([HD, P], BF16, tag="tr")
                    nc.tensor.transpose(trp, norm, id_sb)
                    nc.vector.tensor_copy(
                        out=att_sb[h2 * HD:(h2 + 1) * HD, pt,
                                   ts(2 * pib + it2, P)],
                        in_=trp,
                    )
                    # the h2=1 flush is the last write this i-tile needs for
                    # pair 3 -> its output projections become available
                    if pt == 3 and h2 == 1:
                        pieces.extend((2 * pib + it2, ob) for ob in range(4))

                # output projections run only during pair-3 steps and the
                # tail, after the last qk_half (step (2,7)) -- so they share
                # the rotary's psum bank (psqr, tag "qr") instead of
                # occupying an 8th bank, which went to av double-buffering.
                proj_pool = [psqr]

                def proj_piece():
                    it, ob = pieces.pop(0)
                    op = proj_pool[0].tile([P, 512], F32, tag="qr")
                    for ct in range(4):
                        nc.tensor.matmul(
                            op,
                            att_sb[:, ct, ts(it, P)],
                            wot_sb[:, ct, ts(ob, 512)],
                            start=(ct == 0),
                            stop=(ct == 3),
                        )
                    o_sb = evp.tile([P, 512], F32, tag="osb")
                    nc.vector.tensor_copy(out=o_sb, in_=op)
                    nc.gpsimd.dma_start(
                        out=out_part[ts(it, P), ts(ob, 512)], in_=o_sb
                    )

                def run_half():
                    pair, nb, qk = halves.pop(0)
                    if qk == 0:
                        # prefetch the next n-block (or next pass) while this
                        # one computes
                        if nb + 1 < NNB:
                            pref[(pair, nb + 1)] = load_xcs(nb + 1)
                        elif pair < 3:
                            pref[(pair + 1, 0)] = load_xcs(0)
                            load_wqk(pair + 1)
                    qk_half(pair, nb, qk)
                    if qk == 1:
                        del pref[(pair, nb)]

                def work_item(slot):
                    if slot % 2 == 0 and halves:
                        run_half()
                    elif pieces:
                        proj_piece()
                        if not halves and pieces:
                            proj_piece()
                    elif halves:
                        run_half()

                steps = [(t, ib) for t in range(4) for ib in range(NIB)]
                sc_pools = tc.tile_pool(name="pssc", bufs=2, space="PSUM")
                pssc = sc_pools.__enter__()
                for t, ib in steps:
                    isl = ts(ib, IB)
                    qts = qk_sb[:, t, :]
                    kts = qk_sb[:, 4 + t, :]
                    if t < 3:
                        halves.append((t + 1, ib // 2, ib % 2))
                    expq = []
                    for g in range(8):
                        exp_q = expp.tile([P, 2, 2, IB], BF16, tag="exp")
                        expq.append(exp_q)
                        # one 2-bank psum tile per group, double-buffered so
                        # group g+1's matmuls only wait on the ACTIVATE of
                        # g-1. The two heads of a j2 sub-group contract over
                        # disjoint 64-partition bands (row_grp 0 / 64) and
                        # run concurrently on the PE when adjacent in the
                        # stream, so keep them paired. Layout is [h2, j2, i]
                        # (h2 outermost, one full 2KB bank per head): two
                        # CONCURRENT row-band matmuls must not write the same
                        # psum bank -- with [j2, h2, i] at IB=256 the h0/h64
                        # pair lands in two halves of one bank and the device
                        # faults.
                        scq = pssc.tile([P, 2, 2, IB], F32, tag="sc")
                        for j2 in range(2):
                            jt = g * 2 + j2
                            nc.tensor.matmul(
                                scq[:, 0, j2, :],
                                kts[0:HD, ts(jt, P)],
                                qts[0:HD, isl],
                                start=True,
                                stop=True,
                            )
                            nc.tensor.matmul(
                                scq[:, 1, j2, :],
                                kts[HD:P, ts(jt, P)],
                                qts[HD:P, isl],
                                start=True,
                                stop=True,
                            )
                        for j2 in range(2):
                            nc.scalar.activation(
                                out=exp_q[:, :, j2, :],
                                in_=scq[:, :, j2, :],
                                func=EXP_FUNC,
                                scale=SCALE,
                            )
                        if pending_tr:
                            flush_tr()
                        if prev is not None and g % 2 == 0:
                            attnv_chain(g // 2)
                        if g % 2 == 1:
                            work_item(g // 2)
                    prev = (t, ib, expq)

                sc_pools.__exit__(None, None, None)
                # tail: attn@v of the last step, remaining transposes,
                # remaining output projections (double-buffered psum now that
                # the score pool is closed)
                psd2_cm = tc.tile_pool(name="psd2", bufs=2, space="PSUM")
                proj_pool[0] = psd2_cm.__enter__()
                for c in range(4):
                    if pending_tr:
                        flush_tr()
                    attnv_chain(c)
                    if pieces:
                        proj_piece()
                    if pieces:
                        proj_piece()
                while pending_tr:
                    flush_tr()
                while pieces:
                    proj_piece()
                psd2_cm.__exit__(None, None, None)


def _rot_matrix():
    r = np.zeros((HD, HD), dtype=np.float32)
    for d in range(32):
        r[d, d + 32] = -1.0
    for d in range(32, HD):
        r[d, d - 32] = 1.0
    r2 = np.zeros((P, P), dtype=np.float32)
    r2[0:HD, 0:HD] = r
    r2[HD:P, HD:P] = r
    return np.ascontiguousarray(r2.T)


def _core_inputs(x, cos_t, sin_t, W_qkv, W_out, core):
    b, g = divmod(core, 4)
    hs = g * NHC * HD  # feature offset of this head group (512 per group)

    xT = np.ascontiguousarray(x[b].T)  # [c, n]
    xt = np.ascontiguousarray(
        xT.reshape(KT, P, NNB, NB).transpose(2, 1, 0, 3)
    ).astype(NPBF16)

    Wq = W_qkv[hs:hs + 512]
    Wk = W_qkv[CDIM + hs:CDIM + hs + 512]
    Wv = W_qkv[2 * CDIM + hs:2 * CDIM + hs + 512]
    WqkT = np.ascontiguousarray(np.concatenate([Wq, Wk], axis=0).T)  # [c, 1024]
    wqkt = np.ascontiguousarray(
        WqkT.reshape(KT, P, MF, P).transpose(2, 1, 0, 3)
    ).astype(NPBF16)
    WvT = np.ascontiguousarray(Wv.T)  # [c, 512]
    wvt = np.ascontiguousarray(
        WvT.reshape(KT, P, 512).transpose(1, 0, 2)).astype(NPBF16)
    WoT = np.ascontiguousarray(W_out[:, hs:hs + 512].T)  # [c-slice 512, o 2048]
    wot = np.ascontiguousarray(
        WoT.reshape(4, P, CDIM).transpose(1, 0, 2)).astype(NPBF16)

    return {
        "xt": xt,
        "wqkt": wqkt,
        "wvt": wvt,
        "wot": wot,
        "cos2t": cos_t,
        "sin2t": sin_t,
        "r2t": _ROT,
        "ident": _IDENT,
    }


_ROT = _rot_matrix().astype(np.float16)
_IDENT = np.eye(P, dtype=np.float32).astype(NPBF16)


def kernel(x, freqs, W_qkv, W_out):
    global _CACHED_NC
    x = np.asarray(x, dtype=np.float32)
    freqs = np.asarray(freqs, dtype=np.float32)
    W_qkv = np.asarray(W_qkv, dtype=np.float32)
    W_out = np.asarray(W_out, dtype=np.float32)

    if _CACHED_NC is None:
        _CACHED_NC = _build_nc()
    nc = _CACHED_NC

    cos_t = np.ascontiguousarray(
        np.tile(np.cos(freqs.T), (2, 1))).astype(NPBF16)  # [128, n]
    sin_t = np.ascontiguousarray(
        np.tile(np.sin(freqs.T), (2, 1))).astype(NPBF16)

    in_maps = [
        _core_inputs(x, cos_t, sin_t, W_qkv, W_out, core) for core in range(8)
    ]
    trace = os.environ.get("ATT_TRACE") == "1"
    res = run_bass_kernel_spmd(nc, in_maps, core_ids=list(range(8)), trace=trace)
    if trace and res.exec_time_ns is not None:
        print(f"HW exec time: {res.exec_time_ns} ns")

    out = np.empty((2, NSEQ, CDIM), dtype=np.float32)
    for b in range(2):
        acc = np.zeros((NSEQ, CDIM), dtype=np.float64)
        for g in range(4):
            acc += res.results[4 * b + g]["out_part"]
        out[b] = acc.astype(np.float32)
    return out



# revision 31
# speedup vs baseline: 1.2320x; 1.0267x over previous
"""Trainium2 Bass kernel for the fused attention module.

8-core sharding: data-parallel over batch (B=2) x tensor-parallel over head
groups (32 heads -> 4 groups of 8). Core c handles batch c//4, head group c%4.
Each core computes QKV projection (its head slice), RoPE, full non-causal
attention for its 8 heads, and a partial output projection against its
W_out column slice; the host sums the 4 partials per batch.

v2 design (vs baseline):
- all matmul operands in bf16 (same PE rate as f32r at N>=256 per the cost
  model, half the DMA/SBUF); rotary math stays f32, one bf16 rounding on the
  stored q/k
- q/k stay resident in SBUF ([128c, 8mf, 2048n] bf16) -- no DRAM spill
- attn@v computed transposed: out[i, d] via lhsT=exp[j, i-tile], rhs=v[j, 65]
  (M=128 instead of 65 -> half the PE time); the 65th v column of ones gives
  the softmax denominator; normalize = per-partition reciprocal broadcast;
  PE-transpose (identity matmul) back to [d, i] for the output projection
- phase CD is software-pipelined ("weave"): per head-pair step, the 8
  score-groups (PE) + exps (Act) are interleaved with the previous pair's
  attn@v chains and the previous i-block's output-projection pieces so the
  Activation engine (CD bottleneck ~255us of exp) never starves.

v3 (vs v2):
- scores exploit PE row tiling: the two heads of a pair contract over
  disjoint 64-partition bands (row_grp h0/h64), and the hardware runs
  such matmuls CONCURRENTLY when adjacent in the engine stream (~3ns
  stagger). v2 separated them because sc0/sc1 had separate psum pools
  freed by two sequential ACTIVATEs; v3 puts all 4 score matmuls of a
  group in ONE 4-bank psum tile consumed by ONE ACTIVATE, so all four
  become schedule-ready together and pair up h0/h64 back-to-back.
- rotary rotate-half matmul in bf16 (was f32r at ~2.7x the cycles); the
  permutation matrix is exact in bf16, input rounding is ~0.4%.
- startup: first-needed DMAs (wqk pair 0 chunked, x block 0) issue ahead
  of the ident/rot-matrix loads so the first projection matmul starts
  sooner.

Orientation notes (PE computes out = lhsT.T @ rhs, contraction on partitions):
- qT/kT produced as [f, n] (lhsT = W slice pre-transposed on host, rhs = xT)
- v produced as [n, f] (lhsT = xT tile, rhs = WvT)
- scoresT[j, i] per head (lhsT = kT j-tile, rhs = qT i-block)
- RoPE rotate_half is a partition shift via a constant permutation matmul
"""

import os
import sys

sys.path.insert(0, "/opt/trn_rl_repo")

import numpy as np
import ml_dtypes

import concourse.bass as bass  # noqa: F401
import concourse.mybir as mybir
import concourse.tile as tile
from concourse import bacc
from concourse.bass import ts
from concourse.bass_utils import run_bass_kernel_spmd

F32 = mybir.dt.float32
F32R = mybir.dt.float32r
F16 = mybir.dt.float16
BF16 = mybir.dt.bfloat16
NPBF16 = ml_dtypes.bfloat16

P = 128
NSEQ = 2048          # sequence length
CDIM = 2048          # model dim
HD = 64              # head dim
NHC = 8              # heads per core
KT = CDIM // P       # 16 contraction tiles
NB = 512             # n-block in the fused projection phase
NNB = NSEQ // NB     # 4
IB = 256             # i-block in attention
NIB = NSEQ // IB     # 8
JT = NSEQ // P       # 16 j-tiles
MF = 8               # q/k feature tiles (0-3 q, 4-7 k)
EXP_FUNC = mybir.ActivationFunctionType.Exp
SCALE = 1.0 / 8.0    # 1/sqrt(HD)

_CACHED_NC = None


def _build_nc():
    nc = bacc.Bacc(None)

    xt = nc.declare_dram_parameter("xt", [NNB, P, KT, NB], BF16, isOutput=False)
    wqkt = nc.declare_dram_parameter("wqkt", [MF, P, KT, P], BF16, isOutput=False)
    wvt = nc.declare_dram_parameter("wvt", [P, KT, 512], BF16, isOutput=False)
    wot = nc.declare_dram_parameter("wot", [P, 4, CDIM], BF16, isOutput=False)
    cos2t = nc.declare_dram_parameter("cos2t", [P, NSEQ], BF16, isOutput=False)
    sin2t = nc.declare_dram_parameter("sin2t", [P, NSEQ], BF16, isOutput=False)
    r2t = nc.declare_dram_parameter("r2t", [P, P], F16, isOutput=False)
    ident = nc.declare_dram_parameter("ident", [P, P], BF16, isOutput=False)
    out_part = nc.declare_dram_parameter("out_part", [NSEQ, CDIM], F32, isOutput=True)

    n_repeat = int(os.environ.get("ATT_REPEAT", "1"))

    with tile.TileContext(nc) as tc, nc.allow_low_precision("bf16 matmul kernel"):
        for _rep in range(n_repeat):
            _kernel_body(nc, tc, xt, wqkt, wvt, wot, cos2t, sin2t, r2t,
                         ident, out_part)

    nc.compile()
    return nc


def _kernel_body(nc, tc, xt, wqkt, wvt, wot, cos2t, sin2t, r2t, ident,
                 out_part):
    with tc.tile_pool(name="persist", bufs=1) as persist:
        qk_sb = persist.tile([P, MF, NSEQ], BF16)
        v1_sb = persist.tile([P, JT, NHC, HD + 1], BF16)
        att_sb = persist.tile([P, 4, NSEQ], BF16)
        wot_sb = persist.tile([P, 4, CDIM], BF16)
        id_sb = persist.tile([P, P], BF16)
        r2_sb = persist.tile([P, P], F16)

        # pools that span pass 0 AND the weave (q/k projection streaming)
        with tc.tile_pool(name="wqkp", bufs=2) as wqkp, \
             tc.tile_pool(name="xtp", bufs=2) as xtp, \
             tc.tile_pool(name="csp", bufs=2) as csp, \
             tc.tile_pool(name="stg", bufs=3) as stg, \
             tc.tile_pool(name="psqr", bufs=1, space="PSUM") as psqr:

            wqk_tiles = {}

            def load_wqk(pair):
                wqk_t = wqkp.tile([P, KT, 2, P], BF16, tag="wqk")
                nc.sync.dma_start(out=wqk_t[:, :, 0, :], in_=wqkt[pair])
                nc.sync.dma_start(out=wqk_t[:, :, 1, :], in_=wqkt[4 + pair])
                wqk_tiles[pair] = wqk_t

            pref = {}

            def load_xcs(nb, first=None):
                if first is not None:
                    xt_t = first
                else:
                    xt_t = xtp.tile([P, KT, NB], BF16, tag="xt")
                    nc.sync.dma_start(out=xt_t, in_=xt[nb])
                cos_sb = csp.tile([P, NB], BF16, tag="cos")
                sin_sb = csp.tile([P, NB], BF16, tag="sin")
                nc.sync.dma_start(out=cos_sb, in_=cos2t[:, ts(nb, NB)])
                nc.sync.dma_start(out=sin_sb, in_=sin2t[:, ts(nb, NB)])
                return xt_t, cos_sb, sin_sb

            def qk_half(pair, nb, qk):
                """One rotated q (qk=0) or k (qk=1) projection chain for one
                n-block of head pair `pair`."""
                xt_t, cos_sb, sin_sb = pref[(pair, nb)]
                nsl = ts(nb, NB)
                mf = pair + 4 * qk
                qp = psqr.tile([P, NB], F32, tag="qr")
                for kc in range(KT):
                    nc.tensor.matmul(
                        qp,
                        wqk_tiles[pair][:, kc, qk, :],
                        xt_t[:, kc, :],
                        start=(kc == 0),
                        stop=(kc == KT - 1),
                    )
                qa = stg.tile([P, NB], F16, tag="qa")
                nc.vector.tensor_copy(out=qa, in_=qp)
                rp = psqr.tile([P, NB], F32, tag="qr")
                nc.tensor.matmul(rp, r2_sb, qa, start=True, stop=True)
                # t1/add touch only SBUF -> run on the idle gpsimd engine so
                # the Vector FIFO stays short for the attn@v psum evacuation
                # (Vector keeps the two psum readers: qa cast + t2).
                t1 = stg.tile([P, NB], F32, tag="t1")
                nc.gpsimd.tensor_mul(out=t1, in0=qa, in1=cos_sb)
                t2 = stg.tile([P, NB], F32, tag="t2")
                nc.vector.tensor_mul(out=t2, in0=rp, in1=sin_sb)
                nc.gpsimd.tensor_add(out=qk_sb[:, mf, nsl], in0=t1, in1=t2)

            # ---- pass 0 (serial): v projection + head pair 0 ----
            with tc.tile_pool(name="wvtp", bufs=1) as wvtp, \
                 tc.tile_pool(name="psv", bufs=2, space="PSUM") as psv:
                wvt_sb = wvtp.tile([P, KT, 512], BF16)
                xt_first = xtp.tile([P, KT, NB], BF16, tag="xt")
                xt0_r = xt[0]
                # startup order: interleave the pair-0 q-weight chunks with
                # the first x block so the first projection chain can start
                # after ~256KB instead of the full 1.5MB; then cos/sin (used
                # right after the chain), the rotate matrix, the k-weight
                # chunks, and only then the ident (not needed until the
                # weave). wvt streams behind on the gpsimd queue.
                wqk_t0 = wqkp.tile([P, KT, 2, P], BF16, tag="wqk")
                for c4 in range(0, KT, 4):
                    nc.sync.dma_start(
                        out=wqk_t0[:, c4:c4 + 4, 0, :],
                        in_=wqkt[0][:, c4:c4 + 4, :])
                    for kc in range(c4, c4 + 4):
                        nc.sync.dma_start(out=xt_first[:, kc, :],
                                          in_=xt0_r[:, kc, :])
                    if c4 == 0:
                        pref[(0, 0)] = load_xcs(0, xt_first)
                        nc.sync.dma_start(out=r2_sb, in_=r2t[:, :])
                for c4 in range(0, KT, 4):
                    nc.sync.dma_start(
                        out=wqk_t0[:, c4:c4 + 4, 1, :],
                        in_=wqkt[4][:, c4:c4 + 4, :])
                wqk_tiles[0] = wqk_t0
                nc.sync.dma_start(out=id_sb, in_=ident[:, :])
                nc.vector.memset(v1_sb[:, :, :, HD:HD + 1], 1.0)
                for kc in range(KT):
                    nc.gpsimd.dma_start(out=wvt_sb[:, kc, :], in_=wvt[:, kc, :])
                for nb in range(NNB):
                    if nb > 0:
                        pref[(0, nb)] = load_xcs(nb)
                    xt_t = pref[(0, nb)][0]
                    qk_half(0, nb, 0)
                    qk_half(0, nb, 1)
                    for t4 in range(NB // P):
                        nt = nb * (NB // P) + t4
                        vp = psv.tile([P, 512], F32, tag="vp")
                        for kc in range(KT):
                            nc.tensor.matmul(
                                vp,
                                xt_t[:, kc, ts(t4, P)],
                                wvt_sb[:, kc, :],
                                start=(kc == 0),
                                stop=(kc == KT - 1),
                            )
                        nc.scalar.copy(
                            out=v1_sb[:, nt, :, 0:HD],
                            in_=vp.rearrange("p (h d) -> p h d", h=NHC),
                        )
                    del pref[(0, nb)]

            for ct in range(4):
                nc.sync.dma_start(out=wot_sb[:, ct, :], in_=wot[:, ct, :])
            load_wqk(1)
            pref[(1, 0)] = load_xcs(0)

            # ---- weave: attention steps (t outer, ib inner) with the q/k
            # projections for pair t+1 and the output projections of
            # completed i-blocks interleaved ----
            with tc.tile_pool(name="expp", bufs=18) as expp, \
                 tc.tile_pool(name="normp", bufs=6) as normp, \
                 tc.tile_pool(name="rdp", bufs=6) as rdp, \
                 tc.tile_pool(name="evp", bufs=3) as evp, \
                 tc.tile_pool(name="psavt", bufs=1, space="PSUM") as psavt, \
                 tc.tile_pool(name="psav2", bufs=2, space="PSUM") as psav2:

                prev = None          # (t_pair, ib, expq) of the previous step
                pending_tr = []      # [(norm_tile, h2, t_pair, i_slice), ...]
                halves = []          # [(pair, nb, qk), ...] projection work
                pieces = []          # [(it, ob), ...] output-projection work

                def attnv_chain(c):
                    pt, pib, pexpq = prev
                    h2, it2 = divmod(c, 2)
                    h = 2 * pt + h2
                    # double-buffered: the next chain's accumulation does not
                    # wait for this chain's Vector evacuation (recip + mul)
                    av = psav2.tile([P, P], F32, tag="av")
                    for jt in range(JT):
                        nc.tensor.matmul(
                            av[:, 0:HD + 1],
                            pexpq[jt // 2][:, h2, jt % 2, ts(it2, P)],
                            v1_sb[:, jt, h, :],
                            start=(jt == 0),
                            stop=(jt == JT - 1),
                        )
                    rd = rdp.tile([P, 1], F32, tag="rd")
                    nc.vector.reciprocal(out=rd, in_=av[:, HD:HD + 1])
                    norm = normp.tile([P, HD], BF16, tag="norm")
                    nc.vector.tensor_scalar_mul(norm, av[:, 0:HD], rd)
                    pending_tr.append((norm, h2, pt, pib, it2))

                def flush_tr():
                    norm, h2, pt, pib, it2 = pending_tr.pop(0)
                    trp = psavt.tile([HD, P], BF16, tag="tr")
                    nc.tensor.transpose(trp, norm, id_sb)
                    nc.vector.tensor_copy(
                        out=att_sb[h2 * HD:(h2 + 1) * HD, pt,
                                   ts(2 * pib + it2, P)],
                        in_=trp,
                    )
                    # the h2=1 flush is the last write this i-tile needs for
                    # pair 3 -> its output projections become available
                    if pt == 3 and h2 == 1:
                        pieces.extend((2 * pib + it2, ob) for ob in range(4))

                # output projections run only during pair-3 steps and the
                # tail, after the last qk_half (step (2,7)) -- so they share
                # the rotary's psum bank (psqr, tag "qr") instead of
                # occupying an 8th bank, which went to av double-buffering.
                proj_pool = [psqr]

                piece_no = [0]

                def proj_piece():
                    it, ob = pieces.pop(0)
                    op = proj_pool[0].tile([P, 512], F32, tag="qr")
                    for ct in range(4):
                        nc.tensor.matmul(
                            op,
                            att_sb[:, ct, ts(it, P)],
                            wot_sb[:, ct, ts(ob, 512)],
                            start=(ct == 0),
                            stop=(ct == 3),
                        )
                    o_sb = evp.tile([P, 512], F32, tag="osb")
                    # alternate the psum evacuation between Vector and Scalar
                    # so neither FIFO serializes consecutive pieces
                    if piece_no[0] % 2 == 0:
                        nc.vector.tensor_copy(out=o_sb, in_=op)
                    else:
                        nc.scalar.copy(out=o_sb, in_=op)
                    piece_no[0] += 1
                    nc.gpsimd.dma_start(
                        out=out_part[ts(it, P), ts(ob, 512)], in_=o_sb
                    )

                def run_half():
                    pair, nb, qk = halves.pop(0)
                    if qk == 0:
                        # prefetch the next n-block (or next pass) while this
                        # one computes
                        if nb + 1 < NNB:
                            pref[(pair, nb + 1)] = load_xcs(nb + 1)
                        elif pair < 3:
                            pref[(pair + 1, 0)] = load_xcs(0)
                            load_wqk(pair + 1)
                    qk_half(pair, nb, qk)
                    if qk == 1:
                        del pref[(pair, nb)]

                def work_item(slot):
                    if slot % 2 == 0 and halves:
                        run_half()
                    elif pieces:
                        proj_piece()
                        if not halves and pieces:
                            proj_piece()
                    elif halves:
                        run_half()

                steps = [(t, ib) for t in range(4) for ib in range(NIB)]
                sc_pools = tc.tile_pool(name="pssc", bufs=2, space="PSUM")
                pssc = sc_pools.__enter__()
                for t, ib in steps:
                    isl = ts(ib, IB)
                    qts = qk_sb[:, t, :]
                    kts = qk_sb[:, 4 + t, :]
                    if t < 3:
                        halves.append((t + 1, ib // 2, ib % 2))
                    expq = []
                    for g in range(8):
                        exp_q = expp.tile([P, 2, 2, IB], BF16, tag="exp")
                        expq.append(exp_q)
                        # one 2-bank psum tile per group, double-buffered so
                        # group g+1's matmuls only wait on the ACTIVATE of
                        # g-1. The two heads of a j2 sub-group contract over
                        # disjoint 64-partition bands (row_grp 0 / 64) and
                        # run concurrently on the PE when adjacent in the
                        # stream, so keep them paired. Layout is [h2, j2, i]
                        # (h2 outermost, one full 2KB bank per head): two
                        # CONCURRENT row-band matmuls must not write the same
                        # psum bank -- with [j2, h2, i] at IB=256 the h0/h64
                        # pair lands in two halves of one bank and the device
                        # faults.
                        scq = pssc.tile([P, 2, 2, IB], F32, tag="sc")
                        for j2 in range(2):
                            jt = g * 2 + j2
                            nc.tensor.matmul(
                                scq[:, 0, j2, :],
                                kts[0:HD, ts(jt, P)],
                                qts[0:HD, isl],
                                start=True,
                                stop=True,
                            )
                            nc.tensor.matmul(
                                scq[:, 1, j2, :],
                                kts[HD:P, ts(jt, P)],
                                qts[HD:P, isl],
                                start=True,
                                stop=True,
                            )
                        # one ACTIVATE per head: contiguous single-bank psum
                        # read (the per-j2 split read strided across banks,
                        # ~29% slower on the Act engine)
                        for h2 in range(2):
                            nc.scalar.activation(
                                out=exp_q[:, h2, :, :],
                                in_=scq[:, h2, :, :],
                                func=EXP_FUNC,
                                scale=SCALE,
                            )
                        if pending_tr:
                            flush_tr()
                        if prev is not None and g % 2 == 0:
                            attnv_chain(g // 2)
                        if g % 2 == 1:
                            work_item(g // 2)
                    prev = (t, ib, expq)

                sc_pools.__exit__(None, None, None)
                # tail: attn@v of the last step, remaining transposes,
                # remaining output projections (double-buffered psum now that
                # the score pool is closed)
                psd2_cm = tc.tile_pool(name="psd2", bufs=2, space="PSUM")
                proj_pool[0] = psd2_cm.__enter__()
                for c in range(4):
                    if pending_tr:
                        flush_tr()
                    attnv_chain(c)
                    if pieces:
                        proj_piece()
                    if pieces:
                        proj_piece()
                while pending_tr:
                    flush_tr()
                while pieces:
                    proj_piece()
                psd2_cm.__exit__(None, None, None)


def _rot_matrix():
    r = np.zeros((HD, HD), dtype=np.float32)
    for d in range(32):
        r[d, d + 32] = -1.0
    for d in range(32, HD):
        r[d, d - 32] = 1.0
    r2 = np.zeros((P, P), dtype=np.float32)
    r2[0:HD, 0:HD] = r
    r2[HD:P, HD:P] = r
    return np.ascontiguousarray(r2.T)


def _core_inputs(x, cos_t, sin_t, W_qkv, W_out, core):
    b, g = divmod(core, 4)
    hs = g * NHC * HD  # feature offset of this head group (512 per group)

    xT = np.ascontiguousarray(x[b].T)  # [c, n]
    xt = np.ascontiguousarray(
        xT.reshape(KT, P, NNB, NB).transpose(2, 1, 0, 3)
    ).astype(NPBF16)

    Wq = W_qkv[hs:hs + 512]
    Wk = W_qkv[CDIM + hs:CDIM + hs + 512]
    Wv = W_qkv[2 * CDIM + hs:2 * CDIM + hs + 512]
    WqkT = np.ascontiguousarray(np.concatenate([Wq, Wk], axis=0).T)  # [c, 1024]
    wqkt = np.ascontiguousarray(
        WqkT.reshape(KT, P, MF, P).transpose(2, 1, 0, 3)
    ).astype(NPBF16)
    WvT = np.ascontiguousarray(Wv.T)  # [c, 512]
    wvt = np.ascontiguousarray(
        WvT.reshape(KT, P, 512).transpose(1, 0, 2)).astype(NPBF16)
    WoT = np.ascontiguousarray(W_out[:, hs:hs + 512].T)  # [c-slice 512, o 2048]
    wot = np.ascontiguousarray(
        WoT.reshape(4, P, CDIM).transpose(1, 0, 2)).astype(NPBF16)

    return {
        "xt": xt,
        "wqkt": wqkt,
        "wvt": wvt,
        "wot": wot,
        "cos2t": cos_t,
        "sin2t": sin_t,
        "r2t": _ROT,
        "ident": _IDENT,
    }


_ROT = _rot_matrix().astype(np.float16)
_IDENT = np.eye(P, dtype=np.float32).astype(NPBF16)


def kernel(x, freqs, W_qkv, W_out):
    global _CACHED_NC
    x = np.asarray(x, dtype=np.float32)
    freqs = np.asarray(freqs, dtype=np.float32)
    W_qkv = np.asarray(W_qkv, dtype=np.float32)
    W_out = np.asarray(W_out, dtype=np.float32)

    if _CACHED_NC is None:
        _CACHED_NC = _build_nc()
    nc = _CACHED_NC

    cos_t = np.ascontiguousarray(
        np.tile(np.cos(freqs.T), (2, 1))).astype(NPBF16)  # [128, n]
    sin_t = np.ascontiguousarray(
        np.tile(np.sin(freqs.T), (2, 1))).astype(NPBF16)

    in_maps = [
        _core_inputs(x, cos_t, sin_t, W_qkv, W_out, core) for core in range(8)
    ]
    trace = os.environ.get("ATT_TRACE") == "1"
    res = run_bass_kernel_spmd(nc, in_maps, core_ids=list(range(8)), trace=trace)
    if trace and res.exec_time_ns is not None:
        print(f"HW exec time: {res.exec_time_ns} ns")

    out = np.empty((2, NSEQ, CDIM), dtype=np.float32)
    for b in range(2):
        acc = np.zeros((NSEQ, CDIM), dtype=np.float64)
        for g in range(4):
            acc += res.results[4 * b + g]["out_part"]
        out[b] = acc.astype(np.float32)
    return out

